# revision 9
# baseline (speedup 1.0000x reference)
"""Trainium2 Bass kernel for DenseMLPQMatrixDecoder.

Math: per embedding v, a tiny MLP (8->16->16->16) produces logits for a 4x4
rate matrix Q (zero diag -> exp -> row-normalize off-diag -> diag = -1).
The reference computes expm(Q*1000) per (v, s) and takes row 0; with
T_INF=1000 that equals the stationary distribution pi, broadcast along S.

Algorithm (v2 — verified numerically against the reference):
  * E = exp(logits) with a -100 diagonal-logit bias (E_diag ~ 0).
  * G = E - diag(rowsum(E)) is a generator with the same tree structure;
    pi_Q[j] proportional to u[j]*r[j] where u[j] = (-1)^j det(G del row j, col 0)
    (adjugate-row / Markov tree theorem; the row-normalization of Q cancels).
  * u via one outer product On = G[:,2] x G[:,3], the antisymmetrized
    minor table V[j,r] = +-N[complement(j,r)] (whose 16-cell gather from On
    is GF(2)-linear and splits into THREE affine strided subtracts plus a
    zero diagonal), then four independent scalar_tensor_tensor ops with
    accum_out doing u[j] = sum_r G[r,1]*V[j,r].
  * wr = u*r with accum_out giving the normalizer in the same op.

Sharding: V=1024 split as 128 rows per core across 8 cores (pure data
parallel); MLP weights replicated (same host-packed [17,179] single-DMA
layout as v1, including the ones-row augmented last layer).

Schedule notes:
  * ReLUs run on DVE (tensor_scalar (x+b) max 0 straight out of PSUM),
    keeping the scalar engine free for the exp table + exp + its DMA ring.
  * The [VP,1024] replication is split DVE/ACT halves; the 2MB store is
    split into two DMAs issued on the two HWDGE rings (SP + ACT) so
    descriptor generation overlaps.
  * gpsimd runs two of the four det STTs (plain strided APs only --
    stride-0 broadcast APs crash the gpsimd exec unit on HW).
"""

import numpy as np

import concourse.bacc as bacc
import concourse.bass as bass
import concourse.mybir as mybir
import concourse.tile as tile
from concourse.bass_utils import run_bass_kernel_spmd

AF = mybir.ActivationFunctionType
ALU = mybir.AluOpType
F32 = mybir.dt.float32

V, D, WIDTH, A, S = 1024, 8, 16, 4, 1024
NCORES = 8
VP = V // NCORES          # 128 rows per core
FREE = S * A              # 4096 output elems per row

# Packed input layout [PACK_P, PACK_F] f32 (one DMA -> one semaphore):
#   rows 0:16 cols  0:17   W1 padded with a zero 17th column (so the mm2
#                          output row 16 is 0; relu(0 + bias 1.0) = 1 builds
#                          the ones-row for the augmented-bias last layer)
#   rows 0:17 cols 17:33   Wout with bout as row 16
#   rows 0:16 col  33      b0
#   rows 0:17 col  34      b1 with 1.0 at row 16
#   rows 0:8  cols 35:51   W0
#   rows 0:8  cols 51:179  emb shard, pre-transposed to [D, VP]
PACK_P = WIDTH + 1        # 17
PACK_F = 51 + VP          # 179

# Feature toggles (fallbacks for HW quirks)
CFG_DVE_RELU = True       # ReLU via DVE tensor_scalar from PSUM
CFG_GPSIMD = True         # gpsimd does the V-table memset
CFG_ACT_REP = True        # scalar engine writes part of the replication
CFG_SPLIT_DMA = True      # two output DMAs on SP + ACT HWDGE rings
CFG_F32R = False          # single-pass tf32-like matmuls (f32r) — walrus rejects
REP_DVE = 640             # DVE's share of the [VP,1024] replication


def pack_inputs(W0, b0, W1, b1, Wout, bout, emb) -> list[np.ndarray]:
    """Per-core packed input tensors (emb: full [V, D] array)."""
    base = np.zeros((PACK_P, PACK_F), np.float32)
    base[0:WIDTH, 0:16] = W1
    base[0:WIDTH, 17:33] = Wout
    bout_aug = np.asarray(bout, np.float32).copy()
    bout_aug[[0, 5, 10, 15]] -= 100.0
    base[WIDTH, 17:33] = bout_aug
    base[0:WIDTH, 33] = b0
    base[0:WIDTH, 34] = b1
    base[WIDTH, 34] = 1.0
    base[0:D, 35:51] = W0
    packs = []
    for c in range(NCORES):
        p = base.copy()
        p[0:D, 51 : 51 + VP] = emb[c * VP : (c + 1) * VP].T
        packs.append(p)
    return packs


def _ap(t, off, *dims):
    """Strided AP into tile t: partition dim + given [stride, count] dims."""
    base = t[:]
    return bass.AP(tensor=base.tensor, offset=base.offset + off,
                   ap=[base.ap[0]] + [list(d) for d in dims])


def _build_module() -> bass.Bass:
    nc = bacc.Bacc()

    pack_d = nc.declare_dram_parameter("pack", [PACK_P, PACK_F], F32, isOutput=False)
    out_d = nc.declare_dram_parameter("out", [VP, FREE], F32, isOutput=True)

    with tile.TileContext(nc) as tc:
        with (
            tc.tile_pool(name="sb", bufs=1) as sb,
            tc.tile_pool(name="ps", bufs=1, space="PSUM") as ps,
        ):
            # Dummy no-dep activation: pulls the ~1.3us ACT_TABLE_LOAD to the
            # head of the kernel, parallel with the input DMA.
            warm = sb.tile([1, 1], F32)
            nc.scalar.activation(warm[:], nc.const_aps.tensor(0.0, (1, 1)), AF.Exp)

            # ---- load everything with ONE dma --------------------------------
            raw = sb.tile([PACK_P, PACK_F], F32)
            nc.sync.dma_start(raw[:], pack_d[:])

            # V minor table scratch; zeroed early (diagonal must be 0, the
            # three affine pieces below write the other 12 cells).
            Vt = sb.tile([VP, A * A], F32)
            (nc.gpsimd if CFG_GPSIMD else nc.vector).memset(Vt[:], 0.0)

            w1_aug = raw[0:WIDTH, 0:17]         # [16,17], col 16 = zeros
            wout_aug = raw[0:PACK_P, 17:33]     # [17,16], row 16 = bout_aug
            w0_sb = raw[0:D, 35:51]
            embT = raw[0:D, 51 : 51 + VP]       # [8,128]
            b0_raw = raw[0:WIDTH, 33:34]
            b1_raw = raw[0:PACK_P, 34:35]

            # ---- MLP in feature-major layout: h_T = W.T @ x_T ----------------
            F32R = mybir.dt.float32r
            mm_in = (lambda ap: ap.bitcast(F32R)) if CFG_F32R else (lambda ap: ap)
            ps1 = ps.tile([WIDTH, VP], F32)
            nc.tensor.matmul(ps1[:], mm_in(w0_sb), mm_in(embT))
            h0 = sb.tile([WIDTH, VP], F32)
            if CFG_DVE_RELU:
                nc.vector.tensor_scalar(h0[:], ps1[:], b0_raw, 0.0, ALU.add, ALU.max)
            else:
                nc.scalar.activation(h0[:], ps1[:], AF.Relu, bias=b0_raw)

            ps2 = ps.tile([PACK_P, VP], F32)    # [17,128]; row 16 = 0
            nc.tensor.matmul(ps2[:], mm_in(w1_aug), mm_in(h0[:]))
            h1a = sb.tile([PACK_P, VP], F32)    # relu rows + ones row 16
            if CFG_DVE_RELU:
                nc.vector.tensor_scalar(h1a[:], ps2[:], b1_raw, 0.0, ALU.add, ALU.max)
            else:
                nc.scalar.activation(h1a[:], ps2[:], AF.Relu, bias=b1_raw)

            # last layer emitted v-major: logq[v,k] = h1a.T @ Wout_aug
            ps3 = ps.tile([VP, A * A], F32)
            nc.tensor.matmul(ps3[:], mm_in(h1a[:]), mm_in(wout_aug))
            E = sb.tile([VP, A * A], F32)       # E = exp(logq + bout), diag ~0
            nc.scalar.activation(E[:], ps3[:], AF.Exp)

            # ---- G = E - diag(r):  r = rowsums, then E[ii] = -r_i -----------
            r = sb.tile([VP, A], F32)
            nc.vector.reduce_sum(
                r[:], E[:].rearrange("p (i j) -> p i j", i=A), axis=mybir.AxisListType.X
            )
            nc.vector.tensor_scalar(_ap(E, 0, (5, 4)), r[:], -1.0, None, ALU.mult)

            # ---- On[x,y] = G[x,2]*G[y,3] (outer of columns 2 and 3) ----------
            On = sb.tile([VP, A * A], F32)
            nc.vector.tensor_tensor(
                On[:].rearrange("p (x y) -> p x y", x=A),
                _ap(E, 2, (4, 4), (0, 4)),
                _ap(E, 3, (0, 4), (4, 4)),
                op=ALU.mult,
            )

            # ---- V[j,r] = +-N[complement(j,r)]: three affine pieces ----------
            # V[j,r] = On[s,t] - On[t,s]; the (j,r)->(s,t) gather is XOR-linear
            # and splits by d = j^r into affine 2x2 strided subtracts.
            # (dst_off, dst_s1, dst_s0, a_off, a_s1, a_s0, b_off, b_s1, b_s0)
            pieces = [
                (1, 10, 3, 11, -10, 3, 14, -10, -3),
                (2, 6, 5, 13, -6, -5, 7, 6, -5),
                (3, 6, 3, 6, 6, -3, 9, -6, 3),
            ]
            for (do, d1, d0, ao, a1, a0, bo, b1_, b0_) in pieces:
                nc.vector.tensor_tensor(
                    _ap(Vt, do, (d1, 2), (d0, 2)),
                    _ap(On, ao, (a1, 2), (a0, 2)),
                    _ap(On, bo, (b1_, 2), (b0_, 2)),
                    op=ALU.subtract,
                )

            # ---- u_j = sum_r G[r,1] * V[j,r]: broadcast mult + grouped reduce
            P = sb.tile([VP, A * A], F32)
            nc.vector.tensor_tensor(
                P[:].rearrange("p (j r) -> p j r", j=A),
                Vt[:].rearrange("p (j r) -> p j r", j=A),
                _ap(E, 1, (0, 4), (4, 4)),      # G col 1 broadcast over j
                op=ALU.mult,
            )
            u = sb.tile([VP, A], F32)
            nc.vector.reduce_sum(
                u[:], P[:].rearrange("p (j r) -> p j r", j=A), axis=mybir.AxisListType.X
            )

            # ---- wr = u*r (+ row sum fused), winv, replicate ----------------
            wr = sb.tile([VP, A], F32)
            wsum = sb.tile([VP, 1], F32)
            nc.vector.scalar_tensor_tensor(
                wr[:], u[:], 1.0, r[:], op0=ALU.mult, op1=ALU.mult,
                accum_out=wsum[:],
            )
            winv = sb.tile([VP, 1], F32)
            nc.vector.reciprocal(winv[:], wsum[:])

            REPW = 1024
            rep = sb.tile([VP, REPW], F32)
            cut = REP_DVE if CFG_ACT_REP else REPW
            nc.vector.tensor_scalar(
                rep[:, 0:cut].rearrange("p (r f) -> p r f", f=A),
                wr[:].unsqueeze(1).broadcast_to((VP, cut // A, A)),
                winv[:], None, ALU.mult,
            )
            if CFG_ACT_REP:
                nc.scalar.activation(
                    rep[:, cut:REPW].rearrange("p (r f) -> p r f", f=A),
                    wr[:].unsqueeze(1).broadcast_to((VP, (REPW - cut) // A, A)),
                    AF.Copy, scale=winv[:],
                )

            # ---- store: repeat-source DMA(s), 4KB inner runs ----------------
            if CFG_SPLIT_DMA:
                h2 = FREE // 2
                nc.sync.dma_start(
                    out_d[:, 0:h2].rearrange("v (r f) -> v r f", f=REPW),
                    rep[:].unsqueeze(1).broadcast_to((VP, h2 // REPW, REPW)),
                )
                nc.scalar.dma_start(
                    out_d[:, h2:FREE].rearrange("v (r f) -> v r f", f=REPW),
                    rep[:].unsqueeze(1).broadcast_to((VP, h2 // REPW, REPW)),
                )
            else:
                nc.sync.dma_start(
                    out_d[:].rearrange("v (r f) -> v r f", f=REPW),
                    rep[:].unsqueeze(1).broadcast_to((VP, FREE // REPW, REPW)),
                )

    nc.finalize()
    return nc


_NC_CACHE = None


def _get_module():
    global _NC_CACHE
    if _NC_CACHE is None:
        _NC_CACHE = _build_module()
    return _NC_CACHE


def kernel(**inputs) -> np.ndarray:
    emb = np.ascontiguousarray(np.asarray(inputs["embeddings_VxD"], np.float32))
    packs = pack_inputs(
        *[np.asarray(inputs[k], np.float32) for k in ["W0", "b0", "W1", "b1", "Wout", "bout"]],
        emb,
    )
    nc = _get_module()
    in_maps = [{"pack": packs[c]} for c in range(NCORES)]
    res = run_bass_kernel_spmd(nc, in_maps, list(range(NCORES)))
    out = np.concatenate(
        [res.results[c]["out"].reshape(VP, S, A) for c in range(NCORES)], axis=0
    )
    return out


# revision 13
# speedup vs baseline: 1.0835x; 1.0835x over previous
"""Trainium2 Bass kernel for DenseMLPQMatrixDecoder.

Math: per embedding v, a tiny MLP (8->16->16->16) produces logits for a 4x4
rate matrix Q (zero diag -> exp -> row-normalize off-diag -> diag = -1).
The reference computes expm(Q*1000) per (v, s) and takes row 0; with
T_INF=1000 that equals the stationary distribution pi, broadcast along S.

Algorithm (v2 — verified numerically against the reference):
  * E = exp(logits) with a -100 diagonal-logit bias (E_diag ~ 0).
  * G = E - diag(rowsum(E)) is a generator with the same tree structure;
    pi_Q[j] proportional to u[j]*r[j] where u[j] = (-1)^j det(G del row j, col 0)
    (adjugate-row / Markov tree theorem; the row-normalization of Q cancels).
  * u via one outer product On = G[:,2] x G[:,3], the antisymmetrized
    minor table V[j,r] = +-N[complement(j,r)] (whose 16-cell gather from On
    is GF(2)-linear and splits into THREE affine strided subtracts plus a
    zero diagonal), then four independent scalar_tensor_tensor ops with
    accum_out doing u[j] = sum_r G[r,1]*V[j,r].
  * wr = u*r with accum_out giving the normalizer in the same op.

Sharding: V=1024 split as 128 rows per core across 8 cores (pure data
parallel); MLP weights replicated (same host-packed [17,179] single-DMA
layout as v1, including the ones-row augmented last layer).

Schedule notes:
  * ReLUs run on DVE (tensor_scalar (x+b) max 0 straight out of PSUM),
    keeping the scalar engine free for the exp table + exp + its DMA ring.
  * The [VP,1024] replication is split DVE/ACT halves; the 2MB store is
    split into two DMAs issued on the two HWDGE rings (SP + ACT) so
    descriptor generation overlaps.
  * gpsimd runs two of the four det STTs (plain strided APs only --
    stride-0 broadcast APs crash the gpsimd exec unit on HW).
"""

import numpy as np

import concourse.bacc as bacc
import concourse.bass as bass
import concourse.mybir as mybir
import concourse.tile as tile
from concourse.bass_utils import run_bass_kernel_spmd

AF = mybir.ActivationFunctionType
ALU = mybir.AluOpType
F32 = mybir.dt.float32

V, D, WIDTH, A, S = 1024, 8, 16, 4, 1024
NCORES = 8
VP = V // NCORES          # 128 rows per core
FREE = S * A              # 4096 output elems per row

# Packed input layout [PACK_P, PACK_F] f32 (one DMA -> one semaphore):
#   rows 0:16 cols  0:17   W1 padded with a zero 17th column (so the mm2
#                          output row 16 is 0; relu(0 + bias 1.0) = 1 builds
#                          the ones-row for the augmented-bias last layer)
#   rows 0:17 cols 17:33   Wout with bout as row 16
#   rows 0:16 col  33      b0
#   rows 0:17 col  34      b1 with 1.0 at row 16
#   rows 0:8  cols 35:51   W0
#   rows 0:8  cols 51:179  emb shard, pre-transposed to [D, VP]
PACK_P = WIDTH + 1        # 17
PACK_F = 51 + VP          # 179

# Feature toggles (fallbacks for HW quirks)
CFG_DVE_RELU = True       # ReLU via DVE tensor_scalar from PSUM
CFG_GPSIMD = True         # gpsimd does the V-table memset
CFG_ACT_REP = True        # scalar engine writes part of the replication
CFG_SPLIT_DMA = True      # two output DMAs on SP + ACT HWDGE rings
CFG_F32R = False          # single-pass tf32-like matmuls (f32r) — walrus rejects
REP_DVE = 640             # DVE's share of the [VP,1024] replication


def pack_inputs(W0, b0, W1, b1, Wout, bout, emb) -> list[np.ndarray]:
    """Per-core packed input tensors (emb: full [V, D] array)."""
    base = np.zeros((PACK_P, PACK_F), np.float32)
    base[0:WIDTH, 0:16] = W1
    base[0:WIDTH, 17:33] = Wout
    bout_aug = np.asarray(bout, np.float32).copy()
    bout_aug[[0, 5, 10, 15]] -= 100.0
    base[WIDTH, 17:33] = bout_aug
    base[0:WIDTH, 33] = b0
    base[0:WIDTH, 34] = b1
    base[WIDTH, 34] = 1.0
    base[0:D, 35:51] = W0
    packs = []
    for c in range(NCORES):
        p = base.copy()
        p[0:D, 51 : 51 + VP] = emb[c * VP : (c + 1) * VP].T
        packs.append(p)
    return packs


def _ap(t, off, *dims):
    """Strided AP into tile t: partition dim + given [stride, count] dims."""
    base = t[:]
    return bass.AP(tensor=base.tensor, offset=base.offset + off,
                   ap=[base.ap[0]] + [list(d) for d in dims])


def _build_module() -> bass.Bass:
    nc = bacc.Bacc()

    pack_d = nc.declare_dram_parameter("pack", [PACK_P, PACK_F], F32, isOutput=False)
    out_d = nc.declare_dram_parameter("out", [VP, FREE], F32, isOutput=True)

    with tile.TileContext(nc) as tc:
        if True:
            # Raw sbuf/psum allocations (no tile pools): TileContext's shadow
            # memory still tracks deps by AP; skipping pools removes the
            # pool-entry/exit barrier ceremony (~1us of NEFF span).
            _n = iter(range(1000))
            sbt = lambda shape: nc.alloc_sbuf_tensor(f"sb{next(_n)}", shape, F32)
            pst = lambda shape: nc.alloc_psum_tensor(f"ps{next(_n)}", shape, F32)

            # Dummy no-dep activation: pulls the ~1.3us ACT_TABLE_LOAD to the
            # head of the kernel, parallel with the input DMA.
            warm = sbt([1, 1])
            nc.scalar.activation(warm[:], nc.const_aps.tensor(0.0, (1, 1)), AF.Exp)

            # ---- load everything with ONE dma --------------------------------
            raw = sbt([PACK_P, PACK_F])
            nc.sync.dma_start(raw[:], pack_d[:])

            # V minor table scratch; zeroed early (diagonal must be 0, the
            # three affine pieces below write the other 12 cells).
            Vt = sbt([VP, A * A])
            (nc.gpsimd if CFG_GPSIMD else nc.vector).memset(Vt[:], 0.0)

            w1_aug = raw[0:WIDTH, 0:17]         # [16,17], col 16 = zeros
            wout_aug = raw[0:PACK_P, 17:33]     # [17,16], row 16 = bout_aug
            w0_sb = raw[0:D, 35:51]
            embT = raw[0:D, 51 : 51 + VP]       # [8,128]
            b0_raw = raw[0:WIDTH, 33:34]
            b1_raw = raw[0:PACK_P, 34:35]

            # ---- MLP in feature-major layout: h_T = W.T @ x_T ----------------
            F32R = mybir.dt.float32r
            mm_in = (lambda ap: ap.bitcast(F32R)) if CFG_F32R else (lambda ap: ap)
            ps1 = pst([WIDTH, VP])
            nc.tensor.matmul(ps1[:], mm_in(w0_sb), mm_in(embT))
            h0 = sbt([WIDTH, VP])
            if CFG_DVE_RELU:
                nc.vector.tensor_scalar(h0[:], ps1[:], b0_raw, 0.0, ALU.add, ALU.max)
            else:
                nc.scalar.activation(h0[:], ps1[:], AF.Relu, bias=b0_raw)

            ps2 = pst([PACK_P, VP])    # [17,128]; row 16 = 0
            nc.tensor.matmul(ps2[:], mm_in(w1_aug), mm_in(h0[:]))
            h1a = sbt([PACK_P, VP])    # relu rows + ones row 16
            if CFG_DVE_RELU:
                nc.vector.tensor_scalar(h1a[:], ps2[:], b1_raw, 0.0, ALU.add, ALU.max)
            else:
                nc.scalar.activation(h1a[:], ps2[:], AF.Relu, bias=b1_raw)

            # last layer emitted v-major: logq[v,k] = h1a.T @ Wout_aug
            ps3 = pst([VP, A * A])
            nc.tensor.matmul(ps3[:], mm_in(h1a[:]), mm_in(wout_aug))
            E = sbt([VP, A * A])       # E = exp(logq + bout), diag ~0
            nc.scalar.activation(E[:], ps3[:], AF.Exp)

            # ---- G = E - diag(r):  r = rowsums, then E[ii] = -r_i -----------
            r = sbt([VP, A])
            nc.vector.reduce_sum(
                r[:], E[:].rearrange("p (i j) -> p i j", i=A), axis=mybir.AxisListType.X
            )
            nc.vector.tensor_scalar(_ap(E, 0, (5, 4)), r[:], -1.0, None, ALU.mult)

            # ---- On[x,y] = G[x,2]*G[y,3] (outer of columns 2 and 3) ----------
            On = sbt([VP, A * A])
            nc.vector.tensor_tensor(
                On[:].rearrange("p (x y) -> p x y", x=A),
                _ap(E, 2, (4, 4), (0, 4)),
                _ap(E, 3, (0, 4), (4, 4)),
                op=ALU.mult,
            )

            # ---- V[j,r] = +-N[complement(j,r)]: three affine pieces ----------
            # V[j,r] = On[s,t] - On[t,s]; the (j,r)->(s,t) gather is XOR-linear
            # and splits by d = j^r into affine 2x2 strided subtracts.
            # (dst_off, dst_s1, dst_s0, a_off, a_s1, a_s0, b_off, b_s1, b_s0)
            pieces = [
                (1, 10, 3, 11, -10, 3, 14, -10, -3),
                (2, 6, 5, 13, -6, -5, 7, 6, -5),
                (3, 6, 3, 6, 6, -3, 9, -6, 3),
            ]
            for (do, d1, d0, ao, a1, a0, bo, b1_, b0_) in pieces:
                nc.vector.tensor_tensor(
                    _ap(Vt, do, (d1, 2), (d0, 2)),
                    _ap(On, ao, (a1, 2), (a0, 2)),
                    _ap(On, bo, (b1_, 2), (b0_, 2)),
                    op=ALU.subtract,
                )

            # ---- u_j = sum_r G[r,1] * V[j,r]: broadcast mult + grouped reduce
            P = sbt([VP, A * A])
            nc.vector.tensor_tensor(
                P[:].rearrange("p (j r) -> p j r", j=A),
                Vt[:].rearrange("p (j r) -> p j r", j=A),
                _ap(E, 1, (0, 4), (4, 4)),      # G col 1 broadcast over j
                op=ALU.mult,
            )
            u = sbt([VP, A])
            nc.vector.reduce_sum(
                u[:], P[:].rearrange("p (j r) -> p j r", j=A), axis=mybir.AxisListType.X
            )

            # ---- wr = u*r (+ row sum fused), winv, replicate ----------------
            wr = sbt([VP, A])
            wsum = sbt([VP, 1])
            nc.vector.scalar_tensor_tensor(
                wr[:], u[:], 1.0, r[:], op0=ALU.mult, op1=ALU.mult,
                accum_out=wsum[:],
            )
            winv = sbt([VP, 1])
            nc.vector.reciprocal(winv[:], wsum[:])

            REPW = 1024
            rep = sbt([VP, REPW])
            cut = REP_DVE if CFG_ACT_REP else REPW
            nc.vector.tensor_scalar(
                rep[:, 0:cut].rearrange("p (r f) -> p r f", f=A),
                wr[:].unsqueeze(1).broadcast_to((VP, cut // A, A)),
                winv[:], None, ALU.mult,
            )
            if CFG_ACT_REP:
                nc.scalar.activation(
                    rep[:, cut:REPW].rearrange("p (r f) -> p r f", f=A),
                    wr[:].unsqueeze(1).broadcast_to((VP, (REPW - cut) // A, A)),
                    AF.Copy, scale=winv[:],
                )

            # ---- store: repeat-source DMA(s), 4KB inner runs ----------------
            if CFG_SPLIT_DMA:
                h2 = FREE // 2
                nc.sync.dma_start(
                    out_d[:, 0:h2].rearrange("v (r f) -> v r f", f=REPW),
                    rep[:].unsqueeze(1).broadcast_to((VP, h2 // REPW, REPW)),
                )
                nc.scalar.dma_start(
                    out_d[:, h2:FREE].rearrange("v (r f) -> v r f", f=REPW),
                    rep[:].unsqueeze(1).broadcast_to((VP, h2 // REPW, REPW)),
                )
            else:
                nc.sync.dma_start(
                    out_d[:].rearrange("v (r f) -> v r f", f=REPW),
                    rep[:].unsqueeze(1).broadcast_to((VP, FREE // REPW, REPW)),
                )

    nc.finalize()
    return nc


_NC_CACHE = None


def _get_module():
    global _NC_CACHE
    if _NC_CACHE is None:
        _NC_CACHE = _build_module()
    return _NC_CACHE


def kernel(**inputs) -> np.ndarray:
    emb = np.ascontiguousarray(np.asarray(inputs["embeddings_VxD"], np.float32))
    packs = pack_inputs(
        *[np.asarray(inputs[k], np.float32) for k in ["W0", "b0", "W1", "b1", "Wout", "bout"]],
        emb,
    )
    nc = _get_module()
    in_maps = [{"pack": packs[c]} for c in range(NCORES)]
    res = run_bass_kernel_spmd(nc, in_maps, list(range(NCORES)))
    out = np.concatenate(
        [res.results[c]["out"].reshape(VP, S, A) for c in range(NCORES)], axis=0
    )
    return out


# revision 16
# speedup vs baseline: 1.1044x; 1.0194x over previous
"""Trainium2 Bass kernel for DenseMLPQMatrixDecoder.

Math: per embedding v, a tiny MLP (8->16->16->16) produces logits for a 4x4
rate matrix Q (zero diag -> exp -> row-normalize off-diag -> diag = -1).
The reference computes expm(Q*1000) per (v, s) and takes row 0; with
T_INF=1000 that equals the stationary distribution pi, broadcast along S.

Algorithm (v2 — verified numerically against the reference):
  * E = exp(logits) with a -100 diagonal-logit bias (E_diag ~ 0).
  * G = E - diag(rowsum(E)) is a generator with the same tree structure;
    pi_Q[j] proportional to u[j]*r[j] where u[j] = (-1)^j det(G del row j, col 0)
    (adjugate-row / Markov tree theorem; the row-normalization of Q cancels).
  * u via one outer product On = G[:,2] x G[:,3], the antisymmetrized
    minor table V[j,r] = +-N[complement(j,r)] (whose 16-cell gather from On
    is GF(2)-linear and splits into THREE affine strided subtracts plus a
    zero diagonal), then four independent scalar_tensor_tensor ops with
    accum_out doing u[j] = sum_r G[r,1]*V[j,r].
  * wr = u*r with accum_out giving the normalizer in the same op.

Sharding: V=1024 split as 128 rows per core across 8 cores (pure data
parallel); MLP weights replicated (same host-packed [17,179] single-DMA
layout as v1, including the ones-row augmented last layer).

Schedule notes:
  * ReLUs run on DVE (tensor_scalar (x+b) max 0 straight out of PSUM),
    keeping the scalar engine free for the exp table + exp + its DMA ring.
  * The [VP,1024] replication is split DVE/ACT halves; the 2MB store is
    split into two DMAs issued on the two HWDGE rings (SP + ACT) so
    descriptor generation overlaps.
  * gpsimd runs two of the four det STTs (plain strided APs only --
    stride-0 broadcast APs crash the gpsimd exec unit on HW).
"""

import ml_dtypes
import numpy as np

import concourse.bacc as bacc
import concourse.bass as bass
import concourse.mybir as mybir
import concourse.tile as tile
from concourse.bass_utils import run_bass_kernel_spmd

AF = mybir.ActivationFunctionType
ALU = mybir.AluOpType
F32 = mybir.dt.float32
BF16 = mybir.dt.bfloat16
CFG_BF16 = True           # bf16 weights/activations for single-pass matmuls
MMDT = BF16 if CFG_BF16 else F32
NPDT = ml_dtypes.bfloat16 if CFG_BF16 else np.float32

V, D, WIDTH, A, S = 1024, 8, 16, 4, 1024
NCORES = 8
VP = V // NCORES          # 128 rows per core
FREE = S * A              # 4096 output elems per row

# Packed input layout [PACK_P, PACK_F] f32 (one DMA -> one semaphore):
#   rows 0:16 cols  0:17   W1 padded with a zero 17th column (so the mm2
#                          output row 16 is 0; relu(0 + bias 1.0) = 1 builds
#                          the ones-row for the augmented-bias last layer)
#   rows 0:17 cols 17:33   Wout with bout as row 16
#   rows 0:16 cols 34:36   b0 as f32 (2 bf16 slots, bitcast back on device)
#   rows 0:17 cols 36:38   b1 as f32, with 1.0 at row 16
#   rows 0:8  cols 38:54   W0
#   rows 0:8  cols 54:182  emb shard, pre-transposed to [D, VP]
PACK_P = WIDTH + 1        # 17
PACK_F = 54 + VP          # 182 (even: f32 bitcast needs even bf16 row stride)

# Feature toggles (fallbacks for HW quirks)
CFG_DVE_RELU = True       # ReLU via DVE tensor_scalar from PSUM
CFG_GPSIMD = True         # gpsimd does the V-table memset
CFG_ACT_REP = True        # scalar engine writes part of the replication
CFG_SPLIT_DMA = True      # two output DMAs on SP + ACT HWDGE rings
CFG_F32R = False          # single-pass tf32-like matmuls (f32r) — walrus rejects
REP_DVE = 640             # DVE's share of the [VP,1024] replication


def pack_inputs(W0, b0, W1, b1, Wout, bout, emb) -> list[np.ndarray]:
    """Per-core packed input tensors (emb: full [V, D] array)."""
    base = np.zeros((PACK_P, PACK_F), NPDT)
    base[0:WIDTH, 0:16] = W1
    base[0:WIDTH, 17:33] = Wout
    bout_aug = np.asarray(bout, np.float32).copy()
    bout_aug[[0, 5, 10, 15]] -= 100.0
    base[WIDTH, 17:33] = bout_aug
    u16 = base.view(np.uint16)
    u16[0:WIDTH, 34:36] = np.ascontiguousarray(b0, np.float32).view(np.uint16).reshape(WIDTH, 2)
    b1a = np.zeros(PACK_P, np.float32)
    b1a[0:WIDTH] = b1
    b1a[WIDTH] = 1.0
    u16[0:PACK_P, 36:38] = b1a.view(np.uint16).reshape(PACK_P, 2)
    base[0:D, 38:54] = W0
    packs = []
    for c in range(NCORES):
        p = base.copy()
        p[0:D, 54 : 54 + VP] = emb[c * VP : (c + 1) * VP].T.astype(NPDT)
        packs.append(p)
    return packs


def _ap(t, off, *dims):
    """Strided AP into tile t: partition dim + given [stride, count] dims."""
    base = t[:]
    return bass.AP(tensor=base.tensor, offset=base.offset + off,
                   ap=[base.ap[0]] + [list(d) for d in dims])


def _build_module() -> bass.Bass:
    nc = bacc.Bacc()

    pack_d = nc.declare_dram_parameter("pack", [PACK_P, PACK_F], MMDT, isOutput=False)
    out_d = nc.declare_dram_parameter("out", [VP, FREE], F32, isOutput=True)

    with tile.TileContext(nc) as tc:
        if True:
            # Raw sbuf/psum allocations (no tile pools): TileContext's shadow
            # memory still tracks deps by AP; skipping pools removes the
            # pool-entry/exit barrier ceremony (~1us of NEFF span).
            _n = iter(range(1000))
            sbt = lambda shape: nc.alloc_sbuf_tensor(f"sb{next(_n)}", shape, F32)
            pst = lambda shape: nc.alloc_psum_tensor(f"ps{next(_n)}", shape, F32)

            # Dummy no-dep activation: pulls the ~1.3us ACT_TABLE_LOAD to the
            # head of the kernel, parallel with the input DMA.
            warm = sbt([1, 1])
            nc.scalar.activation(warm[:], nc.const_aps.tensor(0.0, (1, 1)), AF.Exp)

            # ---- load everything with ONE dma --------------------------------
            raw = nc.alloc_sbuf_tensor("raw", [PACK_P, PACK_F], MMDT)
            nc.sync.dma_start(raw[:], pack_d[:])

            # V minor table scratch; zeroed early (diagonal must be 0, the
            # three affine pieces below write the other 12 cells).
            Vt = sbt([VP, A * A])
            (nc.gpsimd if CFG_GPSIMD else nc.vector).memset(Vt[:], 0.0)

            w1_aug = raw[0:WIDTH, 0:17]         # [16,17], col 16 = zeros
            wout_aug = raw[0:PACK_P, 17:33]     # [17,16], row 16 = bout_aug
            w0_sb = raw[0:D, 38:54]
            embT = raw[0:D, 54 : 54 + VP]       # [8,128]
            b0_raw = raw[0:WIDTH, 34:36].bitcast(F32)
            b1_raw = raw[0:PACK_P, 36:38].bitcast(F32)

            # ---- MLP in feature-major layout: h_T = W.T @ x_T ----------------
            F32R = mybir.dt.float32r
            mm_in = (lambda ap: ap.bitcast(F32R)) if CFG_F32R else (lambda ap: ap)
            ps1 = pst([WIDTH, VP])
            nc.tensor.matmul(ps1[:], mm_in(w0_sb), mm_in(embT))
            h0 = nc.alloc_sbuf_tensor("h0", [WIDTH, VP], MMDT)
            if CFG_DVE_RELU:
                nc.vector.tensor_scalar(h0[:], ps1[:], b0_raw, 0.0, ALU.add, ALU.max)
            else:
                nc.scalar.activation(h0[:], ps1[:], AF.Relu, bias=b0_raw)

            ps2 = pst([PACK_P, VP])    # [17,128]; row 16 = 0
            nc.tensor.matmul(ps2[:], mm_in(w1_aug), mm_in(h0[:]))
            h1a = nc.alloc_sbuf_tensor("h1a", [PACK_P, VP], MMDT)    # relu rows + ones row 16
            if CFG_DVE_RELU:
                nc.vector.tensor_scalar(h1a[:], ps2[:], b1_raw, 0.0, ALU.add, ALU.max)
            else:
                nc.scalar.activation(h1a[:], ps2[:], AF.Relu, bias=b1_raw)

            # last layer emitted v-major: logq[v,k] = h1a.T @ Wout_aug
            ps3 = pst([VP, A * A])
            nc.tensor.matmul(ps3[:], mm_in(h1a[:]), mm_in(wout_aug))
            E = sbt([VP, A * A])       # E = exp(logq + bout), diag ~0
            nc.scalar.activation(E[:], ps3[:], AF.Exp)

            # ---- G = E - diag(r):  r = rowsums, then E[ii] = -r_i -----------
            r = sbt([VP, A])
            nc.vector.reduce_sum(
                r[:], E[:].rearrange("p (i j) -> p i j", i=A), axis=mybir.AxisListType.X
            )
            nc.vector.tensor_scalar(_ap(E, 0, (5, 4)), r[:], -1.0, None, ALU.mult)

            # ---- On[x,y] = G[x,2]*G[y,3] (outer of columns 2 and 3) ----------
            On = sbt([VP, A * A])
            nc.vector.tensor_tensor(
                On[:].rearrange("p (x y) -> p x y", x=A),
                _ap(E, 2, (4, 4), (0, 4)),
                _ap(E, 3, (0, 4), (4, 4)),
                op=ALU.mult,
            )

            # ---- V[j,r] = +-N[complement(j,r)]: three affine pieces ----------
            # V[j,r] = On[s,t] - On[t,s]; the (j,r)->(s,t) gather is XOR-linear
            # and splits by d = j^r into affine 2x2 strided subtracts.
            # (dst_off, dst_s1, dst_s0, a_off, a_s1, a_s0, b_off, b_s1, b_s0)
            pieces = [
                (1, 10, 3, 11, -10, 3, 14, -10, -3),
                (2, 6, 5, 13, -6, -5, 7, 6, -5),
                (3, 6, 3, 6, 6, -3, 9, -6, 3),
            ]
            for (do, d1, d0, ao, a1, a0, bo, b1_, b0_) in pieces:
                nc.vector.tensor_tensor(
                    _ap(Vt, do, (d1, 2), (d0, 2)),
                    _ap(On, ao, (a1, 2), (a0, 2)),
                    _ap(On, bo, (b1_, 2), (b0_, 2)),
                    op=ALU.subtract,
                )

            # ---- u_j = sum_r G[r,1] * V[j,r]: broadcast mult + grouped reduce
            P = sbt([VP, A * A])
            nc.vector.tensor_tensor(
                P[:].rearrange("p (j r) -> p j r", j=A),
                Vt[:].rearrange("p (j r) -> p j r", j=A),
                _ap(E, 1, (0, 4), (4, 4)),      # G col 1 broadcast over j
                op=ALU.mult,
            )
            u = sbt([VP, A])
            nc.vector.reduce_sum(
                u[:], P[:].rearrange("p (j r) -> p j r", j=A), axis=mybir.AxisListType.X
            )

            # ---- wr = u*r (+ row sum fused), winv, replicate ----------------
            wr = sbt([VP, A])
            wsum = sbt([VP, 1])
            nc.vector.scalar_tensor_tensor(
                wr[:], u[:], 1.0, r[:], op0=ALU.mult, op1=ALU.mult,
                accum_out=wsum[:],
            )
            winv = sbt([VP, 1])
            nc.vector.reciprocal(winv[:], wsum[:])

            REPW = 1024
            rep = sbt([VP, REPW])
            cut = REP_DVE if CFG_ACT_REP else REPW
            nc.vector.tensor_scalar(
                rep[:, 0:cut].rearrange("p (r f) -> p r f", f=A),
                wr[:].unsqueeze(1).broadcast_to((VP, cut // A, A)),
                winv[:], None, ALU.mult,
            )
            if CFG_ACT_REP:
                nc.scalar.activation(
                    rep[:, cut:REPW].rearrange("p (r f) -> p r f", f=A),
                    wr[:].unsqueeze(1).broadcast_to((VP, (REPW - cut) // A, A)),
                    AF.Copy, scale=winv[:],
                )

            # ---- store: repeat-source DMA(s), 4KB inner runs ----------------
            if CFG_SPLIT_DMA:
                h2 = FREE // 2
                nc.sync.dma_start(
                    out_d[:, 0:h2].rearrange("v (r f) -> v r f", f=REPW),
                    rep[:].unsqueeze(1).broadcast_to((VP, h2 // REPW, REPW)),
                )
                nc.scalar.dma_start(
                    out_d[:, h2:FREE].rearrange("v (r f) -> v r f", f=REPW),
                    rep[:].unsqueeze(1).broadcast_to((VP, h2 // REPW, REPW)),
                )
            else:
                nc.sync.dma_start(
                    out_d[:].rearrange("v (r f) -> v r f", f=REPW),
                    rep[:].unsqueeze(1).broadcast_to((VP, FREE // REPW, REPW)),
                )

    nc.finalize()
    return nc


_NC_CACHE = None


def _get_module():
    global _NC_CACHE
    if _NC_CACHE is None:
        _NC_CACHE = _build_module()
    return _NC_CACHE


def kernel(**inputs) -> np.ndarray:
    emb = np.ascontiguousarray(np.asarray(inputs["embeddings_VxD"], np.float32))
    packs = pack_inputs(
        *[np.asarray(inputs[k], np.float32) for k in ["W0", "b0", "W1", "b1", "Wout", "bout"]],
        emb,
    )
    nc = _get_module()
    in_maps = [{"pack": packs[c]} for c in range(NCORES)]
    res = run_bass_kernel_spmd(nc, in_maps, list(range(NCORES)))
    out = np.concatenate(
        [res.results[c]["out"].reshape(VP, S, A) for c in range(NCORES)], axis=0
    )
    return out


# revision 17
# speedup vs baseline: 1.1255x; 1.0190x over previous
"""Trainium2 Bass kernel for DenseMLPQMatrixDecoder.

Math: per embedding v, a tiny MLP (8->16->16->16) produces logits for a 4x4
rate matrix Q (zero diag -> exp -> row-normalize off-diag -> diag = -1).
The reference computes expm(Q*1000) per (v, s) and takes row 0; with
T_INF=1000 that equals the stationary distribution pi, broadcast along S.

Algorithm (v2 — verified numerically against the reference):
  * E = exp(logits) with a -100 diagonal-logit bias (E_diag ~ 0).
  * G = E - diag(rowsum(E)) is a generator with the same tree structure;
    pi_Q[j] proportional to u[j]*r[j] where u[j] = (-1)^j det(G del row j, col 0)
    (adjugate-row / Markov tree theorem; the row-normalization of Q cancels).
  * u via one outer product On = G[:,2] x G[:,3], the antisymmetrized
    minor table V[j,r] = +-N[complement(j,r)] (whose 16-cell gather from On
    is GF(2)-linear and splits into THREE affine strided subtracts plus a
    zero diagonal), then four independent scalar_tensor_tensor ops with
    accum_out doing u[j] = sum_r G[r,1]*V[j,r].
  * wr = u*r with accum_out giving the normalizer in the same op.

Sharding: V=1024 split as 128 rows per core across 8 cores (pure data
parallel); MLP weights replicated (same host-packed [17,179] single-DMA
layout as v1, including the ones-row augmented last layer).

Schedule notes:
  * ReLUs run on DVE (tensor_scalar (x+b) max 0 straight out of PSUM),
    keeping the scalar engine free for the exp table + exp + its DMA ring.
  * The [VP,1024] replication is split DVE/ACT halves; the 2MB store is
    split into two DMAs issued on the two HWDGE rings (SP + ACT) so
    descriptor generation overlaps.
  * gpsimd runs two of the four det STTs (plain strided APs only --
    stride-0 broadcast APs crash the gpsimd exec unit on HW).
"""

import ml_dtypes
import numpy as np

import concourse.bacc as bacc
import concourse.bass as bass
import concourse.mybir as mybir
import concourse.tile as tile
from concourse.bass_utils import run_bass_kernel_spmd

AF = mybir.ActivationFunctionType
ALU = mybir.AluOpType
F32 = mybir.dt.float32
BF16 = mybir.dt.bfloat16
CFG_BF16 = True           # bf16 weights/activations for single-pass matmuls
MMDT = BF16 if CFG_BF16 else F32
NPDT = ml_dtypes.bfloat16 if CFG_BF16 else np.float32

V, D, WIDTH, A, S = 1024, 8, 16, 4, 1024
NCORES = 8
VP = V // NCORES          # 128 rows per core
FREE = S * A              # 4096 output elems per row

# Packed input layout [PACK_P, PACK_F] f32 (one DMA -> one semaphore):
#   rows 0:16 cols  0:17   W1 padded with a zero 17th column (so the mm2
#                          output row 16 is 0; relu(0 + bias 1.0) = 1 builds
#                          the ones-row for the augmented-bias last layer)
#   rows 0:17 cols 17:33   Wout with bout as row 16
#   rows 0:16 cols 34:36   b0 as f32 (2 bf16 slots, bitcast back on device)
#   rows 0:17 cols 36:38   b1 as f32, with 1.0 at row 16
#   rows 0:8  cols 38:54   W0
#   rows 0:8  cols 54:182  emb shard, pre-transposed to [D, VP]
PACK_P = WIDTH + 1        # 17
PACK_F = 54 + VP          # 182 (even: f32 bitcast needs even bf16 row stride)

# Feature toggles (fallbacks for HW quirks)
CFG_DVE_RELU = True       # ReLU via DVE tensor_scalar from PSUM
CFG_GPSIMD = True         # gpsimd does the V-table memset
CFG_ACT_REP = True        # scalar engine writes part of the replication
CFG_SPLIT_DMA = True      # two output DMAs on SP + ACT HWDGE rings
CFG_F32R = False          # single-pass tf32-like matmuls (f32r) — walrus rejects
REP_DVE = 768             # DVE's share of the [VP,1024] replication


def pack_inputs(W0, b0, W1, b1, Wout, bout, emb) -> list[np.ndarray]:
    """Per-core packed input tensors (emb: full [V, D] array)."""
    base = np.zeros((PACK_P, PACK_F), NPDT)
    base[0:WIDTH, 0:16] = W1
    base[0:WIDTH, 17:33] = Wout
    bout_aug = np.asarray(bout, np.float32).copy()
    bout_aug[[0, 5, 10, 15]] -= 100.0
    base[WIDTH, 17:33] = bout_aug
    u16 = base.view(np.uint16)
    u16[0:WIDTH, 34:36] = np.ascontiguousarray(b0, np.float32).view(np.uint16).reshape(WIDTH, 2)
    b1a = np.zeros(PACK_P, np.float32)
    b1a[0:WIDTH] = b1
    b1a[WIDTH] = 1.0
    u16[0:PACK_P, 36:38] = b1a.view(np.uint16).reshape(PACK_P, 2)
    base[0:D, 38:54] = W0
    packs = []
    for c in range(NCORES):
        p = base.copy()
        p[0:D, 54 : 54 + VP] = emb[c * VP : (c + 1) * VP].T.astype(NPDT)
        packs.append(p)
    return packs


def _ap(t, off, *dims):
    """Strided AP into tile t: partition dim + given [stride, count] dims."""
    base = t[:]
    return bass.AP(tensor=base.tensor, offset=base.offset + off,
                   ap=[base.ap[0]] + [list(d) for d in dims])


def _build_module() -> bass.Bass:
    nc = bacc.Bacc()

    pack_d = nc.declare_dram_parameter("pack", [PACK_P, PACK_F], MMDT, isOutput=False)
    out_d = nc.declare_dram_parameter("out", [VP, FREE], F32, isOutput=True)

    with tile.TileContext(nc) as tc:
        if True:
            # Raw sbuf/psum allocations (no tile pools): TileContext's shadow
            # memory still tracks deps by AP; skipping pools removes the
            # pool-entry/exit barrier ceremony (~1us of NEFF span).
            _n = iter(range(1000))
            sbt = lambda shape: nc.alloc_sbuf_tensor(f"sb{next(_n)}", shape, F32)
            pst = lambda shape: nc.alloc_psum_tensor(f"ps{next(_n)}", shape, F32)

            # Dummy no-dep activation: pulls the ~1.3us ACT_TABLE_LOAD to the
            # head of the kernel, parallel with the input DMA.
            warm = sbt([1, 1])
            nc.scalar.activation(warm[:], nc.const_aps.tensor(0.0, (1, 1)), AF.Exp)

            # ---- load everything with ONE dma --------------------------------
            raw = nc.alloc_sbuf_tensor("raw", [PACK_P, PACK_F], MMDT)
            nc.sync.dma_start(raw[:], pack_d[:])

            # V minor table scratch; zeroed early (diagonal must be 0, the
            # three affine pieces below write the other 12 cells).
            Vt = sbt([VP, A * A])
            (nc.gpsimd if CFG_GPSIMD else nc.vector).memset(Vt[:], 0.0)

            w1_aug = raw[0:WIDTH, 0:17]         # [16,17], col 16 = zeros
            wout_aug = raw[0:PACK_P, 17:33]     # [17,16], row 16 = bout_aug
            w0_sb = raw[0:D, 38:54]
            embT = raw[0:D, 54 : 54 + VP]       # [8,128]
            b0_raw = raw[0:WIDTH, 34:36].bitcast(F32)
            b1_raw = raw[0:PACK_P, 36:38].bitcast(F32)

            # ---- MLP in feature-major layout: h_T = W.T @ x_T ----------------
            F32R = mybir.dt.float32r
            mm_in = (lambda ap: ap.bitcast(F32R)) if CFG_F32R else (lambda ap: ap)
            ps1 = pst([WIDTH, VP])
            nc.tensor.matmul(ps1[:], mm_in(w0_sb), mm_in(embT))
            h0 = nc.alloc_sbuf_tensor("h0", [WIDTH, VP], MMDT)
            if CFG_DVE_RELU:
                nc.vector.tensor_scalar(h0[:], ps1[:], b0_raw, 0.0, ALU.add, ALU.max)
            else:
                nc.scalar.activation(h0[:], ps1[:], AF.Relu, bias=b0_raw)

            ps2 = pst([PACK_P, VP])    # [17,128]; row 16 = 0
            nc.tensor.matmul(ps2[:], mm_in(w1_aug), mm_in(h0[:]))
            h1a = nc.alloc_sbuf_tensor("h1a", [PACK_P, VP], MMDT)    # relu rows + ones row 16
            if CFG_DVE_RELU:
                nc.vector.tensor_scalar(h1a[:], ps2[:], b1_raw, 0.0, ALU.add, ALU.max)
            else:
                nc.scalar.activation(h1a[:], ps2[:], AF.Relu, bias=b1_raw)

            # last layer emitted v-major: logq[v,k] = h1a.T @ Wout_aug
            ps3 = pst([VP, A * A])
            nc.tensor.matmul(ps3[:], mm_in(h1a[:]), mm_in(wout_aug))
            E = sbt([VP, A * A])       # E = exp(logq + bout), diag ~0
            nc.scalar.activation(E[:], ps3[:], AF.Exp)

            # ---- G columns 1..3 as [VP, (r,c)] = [VP,12]:  gcols[r,c-1] ------
            # copy of E cols 1..3 runs parallel to the row-sum reduce; the
            # diagonal cells (r=c) are then overwritten with -r_c.
            gcols = sbt([VP, 12])
            nc.vector.tensor_copy(
                gcols[:].rearrange("p (i c) -> p i c", i=A),
                _ap(E, 1, (4, 4), (1, 3)),
            )
            r = sbt([VP, A])
            nc.vector.reduce_sum(
                r[:], E[:].rearrange("p (i j) -> p i j", i=A), axis=mybir.AxisListType.X
            )
            nc.vector.tensor_scalar(_ap(gcols, 3, (4, 3)), r[:, 1:4], -1.0, None, ALU.mult)

            # ---- On[x,y] = G[x,2]*G[y,3] (outer of columns 2 and 3) ----------
            On = sbt([VP, A * A])
            nc.vector.tensor_tensor(
                On[:].rearrange("p (x y) -> p x y", x=A),
                _ap(gcols, 1, (3, 4), (0, 4)),
                _ap(gcols, 2, (0, 4), (3, 4)),
                op=ALU.mult,
            )

            # ---- V[j,r] = +-N[complement(j,r)]: three affine pieces ----------
            # V[j,r] = On[s,t] - On[t,s]; the (j,r)->(s,t) gather is XOR-linear
            # and splits by d = j^r into affine 2x2 strided subtracts.
            # (dst_off, dst_s1, dst_s0, a_off, a_s1, a_s0, b_off, b_s1, b_s0)
            pieces = [
                (1, 10, 3, 11, -10, 3, 14, -10, -3),
                (2, 6, 5, 13, -6, -5, 7, 6, -5),
                (3, 6, 3, 6, 6, -3, 9, -6, 3),
            ]
            for (do, d1, d0, ao, a1, a0, bo, b1_, b0_) in pieces:
                nc.vector.tensor_tensor(
                    _ap(Vt, do, (d1, 2), (d0, 2)),
                    _ap(On, ao, (a1, 2), (a0, 2)),
                    _ap(On, bo, (b1_, 2), (b0_, 2)),
                    op=ALU.subtract,
                )

            # ---- u_j = sum_r G[r,1] * V[j,r]: broadcast mult + grouped reduce
            P = sbt([VP, A * A])
            nc.vector.tensor_tensor(
                P[:].rearrange("p (j r) -> p j r", j=A),
                Vt[:].rearrange("p (j r) -> p j r", j=A),
                _ap(gcols, 0, (0, 4), (3, 4)),  # G col 1 broadcast over j
                op=ALU.mult,
            )
            u = sbt([VP, A])
            nc.vector.reduce_sum(
                u[:], P[:].rearrange("p (j r) -> p j r", j=A), axis=mybir.AxisListType.X
            )

            # ---- wr = u*r (+ row sum fused), winv, replicate ----------------
            wr = sbt([VP, A])
            wsum = sbt([VP, 1])
            nc.vector.scalar_tensor_tensor(
                wr[:], u[:], 1.0, r[:], op0=ALU.mult, op1=ALU.mult,
                accum_out=wsum[:],
            )
            winv = sbt([VP, 1])
            nc.vector.reciprocal(winv[:], wsum[:])

            REPW = 1024
            rep = sbt([VP, REPW])
            cut = REP_DVE if CFG_ACT_REP else REPW
            nc.vector.tensor_scalar(
                rep[:, 0:cut].rearrange("p (r f) -> p r f", f=A),
                wr[:].unsqueeze(1).broadcast_to((VP, cut // A, A)),
                winv[:], None, ALU.mult,
            )
            if CFG_ACT_REP:
                nc.scalar.activation(
                    rep[:, cut:REPW].rearrange("p (r f) -> p r f", f=A),
                    wr[:].unsqueeze(1).broadcast_to((VP, (REPW - cut) // A, A)),
                    AF.Copy, scale=winv[:],
                )

            # ---- store: repeat-source DMA(s), 4KB inner runs ----------------
            if CFG_SPLIT_DMA:
                h2 = FREE // 2
                nc.sync.dma_start(
                    out_d[:, 0:h2].rearrange("v (r f) -> v r f", f=REPW),
                    rep[:].unsqueeze(1).broadcast_to((VP, h2 // REPW, REPW)),
                )
                nc.scalar.dma_start(
                    out_d[:, h2:FREE].rearrange("v (r f) -> v r f", f=REPW),
                    rep[:].unsqueeze(1).broadcast_to((VP, h2 // REPW, REPW)),
                )
            else:
                nc.sync.dma_start(
                    out_d[:].rearrange("v (r f) -> v r f", f=REPW),
                    rep[:].unsqueeze(1).broadcast_to((VP, FREE // REPW, REPW)),
                )

    nc.finalize()
    return nc


_NC_CACHE = None


def _get_module():
    global _NC_CACHE
    if _NC_CACHE is None:
        _NC_CACHE = _build_module()
    return _NC_CACHE


def kernel(**inputs) -> np.ndarray:
    emb = np.ascontiguousarray(np.asarray(inputs["embeddings_VxD"], np.float32))
    packs = pack_inputs(
        *[np.asarray(inputs[k], np.float32) for k in ["W0", "b0", "W1", "b1", "Wout", "bout"]],
        emb,
    )
    nc = _get_module()
    in_maps = [{"pack": packs[c]} for c in range(NCORES)]
    res = run_bass_kernel_spmd(nc, in_maps, list(range(NCORES)))
    out = np.concatenate(
        [res.results[c]["out"].reshape(VP, S, A) for c in range(NCORES)], axis=0
    )
    return out
